# revision 42
# baseline (speedup 1.0000x reference)
"""MultiHeadAttention (B=2, S=2048, D=1024, H=16, depth=64) on 8 trn2 cores.

Sharding: core c -> batch b=c//4, head-group g=c%4 (heads 4g..4g+3).
Per-core device program (SPMD, identical program, different inputs):
  - inputs pre-transposed AND cast to fp16 on host: xq/xk/xv = x_b.T
    [1024, 2048]; weights fp16; biases fp32 column vectors [256, 1].
  - DMA order xv, xk, xq (serial ~360GB/s pipe): V projection first so
    the seq-major V tiles exist before attention starts; K next; Q is
    projected per 512-column q-slice ([128, 8, 512] DMA views), with
    slice 0 / head-pair 0 done up front and the rest emitted INSIDE
    the attention loop (hidden in the ACT exp shadow).
  - V/K projections feature-major, chunk-paced kk-outer over 8 psum
    banks; per-partition bias folded into the DVE psum->SBUF copy.
    DVE copy order: V, K-pch0, Q0-pch0, K-pch1 -- so the first scores
    and the attention psum pools (which reuse K's banks) unblock in
    the order the pipeline needs them.
  - V transposed on-device (PE transpose into the output-projection
    psum slots) and interleaved by DVE into seq-major vI[sc]
    [128, 4, 65] tiles with an all-ones column 64 per head, so attn@V
    also produces the softmax denominator (row 64 of ctx psum).  The
    first 6 seq-blocks are done before the g-loop, the rest ride the
    per-g fill-unit slots of q-slice 0.
  - attention processes head PAIRS with a one-g software-pipeline skew:
    scores(g) for both heads issue before attn@V(g-1); the last attn@V
    + normalization of each head-pair is DEFERRED past the next pair's
    first scores so the ACT exp stream never gaps at boundaries.  exp
    on ACT (scale 1/8, no max subtraction: scores ~ N(0,1)); ACT does
    nothing except exp (table preloaded at t=0).
  - normalization: reciprocal of ctx row 64 (DVE, f32r) -> rank-1 PE
    matmul broadcast -> multiply (DVE) into feature-major fp16 ctxN.
  - output projection of q-slice j spread over fill-unit slots inside
    q-slice j+1's attention; psum -> fp16 SBUF (DVE) in m-block pairs,
    one DMA per pair.  Host sums the 4 head-group partials per batch
    in f32, transposes back, adds bo.
"""

import numpy as np

B, S, D = 2, 2048, 1024
FG = 256  # features per core (4 heads x 64)

_compiled = None


def _build_program(repeat=1):
    import concourse.bass as bass  # noqa: F401
    import concourse.tile as tile
    from concourse import bacc, mybir, masks

    f32 = mybir.dt.float32
    f32r = mybir.dt.float32r
    f16 = mybir.dt.float16
    EXP = mybir.ActivationFunctionType.Exp
    MULT = mybir.AluOpType.mult

    nc = bacc.Bacc("TRN2", target_bir_lowering=False, debug=False)

    xq_d = nc.dram_tensor("xq", [D, S], f16, kind="ExternalInput")
    xk_d = nc.dram_tensor("xk", [D, S], f16, kind="ExternalInput")
    xv_d = nc.dram_tensor("xv", [D, S], f16, kind="ExternalInput")
    wq_d = nc.dram_tensor("wq", [D, FG], f16, kind="ExternalInput")
    wk_d = nc.dram_tensor("wk", [D, FG], f16, kind="ExternalInput")
    wv_d = nc.dram_tensor("wv", [D, FG], f16, kind="ExternalInput")
    wo_d = nc.dram_tensor("wo", [FG, D], f16, kind="ExternalInput")
    bq_d = nc.dram_tensor("bq", [FG, 1], f32, kind="ExternalInput")
    bk_d = nc.dram_tensor("bk", [FG, 1], f32, kind="ExternalInput")
    bv_d = nc.dram_tensor("bv", [FG, 1], f32, kind="ExternalInput")
    out_d = nc.dram_tensor("out", [D, S], f16, kind="ExternalOutput")

    with tile.TileContext(nc) as tc:
      for _rep in range(repeat):
        with tc.tile_pool(name="const", bufs=1) as cpool:
            onesf = cpool.tile([1, 64], f32, tag="onesf", name="onesf")
            nc.gpsimd.memset(onesf[:], 1.0)
            ones_r = cpool.tile([1, 64], f32r, tag="ones_r", name="ones_r")
            nc.vector.tensor_copy(ones_r[:], onesf[:])
            o41f = cpool.tile([128, 4, 1], f32, tag="o41f", name="o41f")
            nc.gpsimd.memset(o41f[:], 1.0)
            ones41 = cpool.tile([128, 4, 1], f16, tag="ones41", name="ones41")
            nc.gpsimd.tensor_copy(ones41[:], o41f[:])
            zbias = cpool.tile([128, 1], f32, tag="zbias", name="zbias")
            nc.gpsimd.memset(zbias[:], 0.0)
            # preload the ACT exp table while projections run
            actwarm = cpool.tile([128, 1], f16, tag="actwarm", name="actwarm")
            nc.scalar.activation(actwarm[:], zbias[:], EXP, bias=zbias[:],
                                 scale=1.0)
            ident = cpool.tile([128, 128], f16, tag="ident", name="ident")
            masks.make_identity(nc, ident[:])
            dumf = cpool.tile([1, 512], f32, tag="dumf", name="dumf")
            nc.gpsimd.memset(dumf[:], 1.0)
            dum_r = cpool.tile([1, 512], f32r, tag="dum_r", name="dum_r")
            nc.vector.tensor_copy(dum_r[:], dumf[:])

            w_sb = {}
            b_sb = {}

            def load_w(nm, d):
                t = cpool.tile([128, 8, FG], f16, tag=nm, name=nm)
                nc.sync.dma_start(t[:], d.ap().rearrange("(k p) f -> p k f", p=128))
                w_sb[nm] = t

            def load_b(nm, d):
                t = cpool.tile([128, 2, 1], f32, tag=nm, name=nm)
                nc.sync.dma_start(t[:], d.ap().rearrange("(k p) o -> p k o", p=128))
                b_sb[nm] = t

            qT = [cpool.tile([128, S], f16, tag=f"qT{p}", name=f"qT{p}")
                  for p in range(2)]
            kT = [cpool.tile([128, S], f16, tag=f"kT{p}", name=f"kT{p}")
                  for p in range(2)]
            vT = [cpool.tile([128, S], f16, tag=f"vT{p}", name=f"vT{p}")
                  for p in range(2)]
            vI = [cpool.tile([128, 4, 65], f16, tag=f"vI{sc}", name=f"vI{sc}")
                  for sc in range(16)]
            for sc in range(16):
                nc.gpsimd.tensor_copy(vI[sc][:, :, 64:65], ones41[:])
            ctxN = [cpool.tile([128, S], f16, tag=f"ctxN{p}", name=f"ctxN{p}")
                    for p in range(2)]

            # ------------- V and K projections (8 psum banks, kk-outer) ------
            with tc.tile_pool(name="xp", bufs=1) as xpool, \
                 tc.tile_pool(name="pp", bufs=1, space="PSUM") as ppool:

                def proj_matmuls(x_d, wname):
                    ps = [ppool.tile([128, 512], f32, name=f"pp{i}", bufs=1)
                          for i in range(8)]
                    xss = []
                    for c in range(4):
                        xs = xpool.tile([128, 2, S], f16, name="xs", bufs=8)
                        nc.sync.dma_start(
                            xs[:],
                            x_d.ap()[256 * c:256 * (c + 1), :]
                                .rearrange("(j p) s -> p j s", p=128))
                        xss.append(xs)
                    for kk in range(8):
                        for pch in range(2):
                            for qc in range(4):
                                i = pch * 4 + qc
                                nc.tensor.matmul(
                                    ps[i][:],
                                    w_sb[wname][:, kk, 128 * pch:128 * (pch + 1)],
                                    xss[kk // 2][:, kk % 2,
                                                 512 * qc:512 * (qc + 1)],
                                    start=(kk == 0), stop=(kk == 7))
                    return ps

                def proj_copies(ps, bname, outT, pch):
                    for qc in range(4):
                        nc.vector.tensor_scalar_add(
                            outT[pch][:, 512 * qc:512 * (qc + 1)],
                            ps[pch * 4 + qc][:], b_sb[bname][:, pch, :])

                def xq_dma(qj):
                    xs = xpool.tile([128, 8, 512], f16, name=f"xq{qj}", bufs=1)
                    nc.sync.dma_start(
                        xs[:],
                        xq_d.ap()[:, 512 * qj:512 * (qj + 1)]
                            .rearrange("(k p) s -> p k s", p=128))
                    return xs

                xqs = []

                def project_q_pch(qj, pch, pool_tile):
                    for kk in range(8):
                        nc.tensor.matmul(
                            pool_tile[:],
                            w_sb["wq"][:, kk, 128 * pch:128 * (pch + 1)],
                            xqs[qj][:, kk, :],
                            start=(kk == 0), stop=(kk == 7))
                    nc.vector.tensor_scalar_add(
                        qT[pch][:, 512 * qj:512 * (qj + 1)],
                        pool_tile[:], b_sb["bq"][:, pch, :])

                # PE p-state warmup: dummy rank-1 matmuls keep the PE
                # ramping until the first xk chunk lands
                warm = ppool.tile([64, 512], f32, name="pp7", bufs=1)
                for _w in range(12):
                    nc.tensor.matmul(warm[:], ones_r[:, 0:64], dum_r[:],
                                     start=True, stop=True)
                # wire: wk,bk, xk, wq,bq, xq0, wv,bv, xv, wo, xq1-3
                load_w("wk", wk_d)
                load_b("bk", bk_d)
                ps = proj_matmuls(xk_d, "wk")
                load_w("wq", wq_d)
                load_b("bq", bq_d)
                xqs.append(xq_dma(0))
                proj_copies(ps, "bk", kT, 0)
                # q-slice 0 / head-pair 0 right after K
                psq = ppool.tile([128, 512], f32, name="pp0", bufs=1)
                project_q_pch(0, 0, psq)
                load_w("wv", wv_d)
                load_b("bv", bv_d)
                proj_copies(ps, "bk", kT, 1)
                vps = proj_matmuls(xv_d, "wv")
                wo_sb = cpool.tile([128, 2, D], f16, tag="wo", name="wo")
                nc.sync.dma_start(wo_sb[:],
                                  wo_d.ap().rearrange("(k p) f -> p k f", p=128))
                for qj in range(1, 4):
                    xqs.append(xq_dma(qj))

                # drain V psum on DVE and ACT in parallel: the attention
                # psum pools wait for the whole 8-bank region
                IDENT_F = mybir.ActivationFunctionType.Identity
                for i, (pch, qc) in enumerate(
                        ((0, 0), (0, 1), (1, 0), (1, 1),
                         (0, 2), (0, 3), (1, 2), (1, 3))):
                    if i % 2 == 0:
                        nc.vector.tensor_scalar_add(
                            vT[pch][:, 512 * qc:512 * (qc + 1)],
                            vps[pch * 4 + qc][:], b_sb["bv"][:, pch, :])
                    else:
                        nc.scalar.activation(
                            vT[pch][:, 512 * qc:512 * (qc + 1)],
                            vps[pch * 4 + qc][:], IDENT_F,
                            bias=b_sb["bv"][:, pch, :], scale=1.0)

            # ---------------- attention + output projection ------------------
            with tc.tile_pool(name="opp", bufs=2, space="PSUM") as opp, \
                 tc.tile_pool(name="exp", bufs=8) as expool, \
                 tc.tile_pool(name="rcp", bufs=2) as rcpool, \
                 tc.tile_pool(name="csp", bufs=2) as cspool, \
                 tc.tile_pool(name="obp", bufs=4) as obpool:

                def emit_outproj_pair(qj, m, pool=None, bufs=2,
                                      use_act=False):
                    pool = pool or opp
                    ops = []
                    for mm in (m, m + 1):
                        op = pool.tile([128, 512], f32, name="op", bufs=bufs)
                        for kk2 in range(2):
                            nc.tensor.matmul(
                                op[:],
                                wo_sb[:, kk2, 128 * mm:128 * (mm + 1)],
                                ctxN[kk2][:, 512 * qj:512 * (qj + 1)],
                                start=(kk2 == 0), stop=(kk2 == 1))
                        ops.append(op)
                    ob = obpool.tile([128, 2, 512], f16, name="ob", bufs=4)
                    for t in range(2):
                        if use_act:
                            nc.scalar.activation(ob[:, t, :], ops[t][:],
                                                 mybir.ActivationFunctionType.Copy,
                                                 bias=0.0, scale=1.0)
                        else:
                            nc.vector.tensor_copy(ob[:, t, :], ops[t][:])
                    nc.sync.dma_start(
                        out_d.ap()[128 * m:128 * (m + 2),
                                   512 * qj:512 * (qj + 1)]
                            .rearrange("(t p) s -> p t s", p=128),
                        ob[:])

                def emit_qproj(qj, pch):
                    pt = opp.tile([128, 512], f32, name="op", bufs=2)
                    project_q_pch(qj, pch, pt)

                def emit_vi(sc, pch):
                    tp = opp.tile([128, 128], f16, name="op", bufs=2)
                    nc.tensor.transpose(
                        tp[:], vT[pch][:, 128 * sc:128 * (sc + 1)],
                        ident[:])
                    nc.vector.tensor_copy(
                        vI[sc][:, 2 * pch:2 * pch + 2, 0:64],
                        tp[:].rearrange("p (h e) -> p h e", h=2))

                # trailing work deferred across head-pair boundaries:
                # phase 0 = last attn@V + ctx staging; phase 1 = normalize
                def trail_p0(st):
                    qj, pch, ctxs, pend, css, rcs = st
                    for hh in range(2):
                        for j in range(2):
                            pk = 14 + j
                            nc.tensor.matmul(
                                ctxs[hh][:], vI[pk][:, 2 * pch + hh, :],
                                pend[hh][:, j, :],
                                start=False, stop=(pk == 15))
                    for hh in range(2):
                        cs = cspool.tile([64, 512], f32, name="cs", bufs=4)
                        nc.vector.tensor_copy(cs[:], ctxs[hh][0:64, :])
                        css.append(cs)
                        rc = rcpool.tile([1, 512], f32r, name="rc", bufs=4)
                        with nc.allow_low_precision(
                                reason="f32r for PE broadcast"):
                            nc.vector.reciprocal(rc[:], ctxs[hh][64:65, :])
                        rcs.append(rc)

                def trail_p1(st):
                    qj, pch, ctxs, pend, css, rcs = st
                    for hh in range(2):
                        off = 64 * hh
                        bc = opp.tile([64, 512], f32, name="op", bufs=2)
                        nc.tensor.matmul(bc[:], ones_r[:, :], rcs[hh][:],
                                         start=True, stop=True)
                        nc.vector.tensor_tensor(
                            ctxN[pch][off:off + 64, 512 * qj:512 * (qj + 1)],
                            css[hh][:], bc[:], MULT)

                trailing = None
                with tc.tile_pool(name="scp", bufs=1, space="PSUM") as scp, \
                     tc.tile_pool(name="cxp", bufs=1, space="PSUM") as cxp:
                  for qj in range(4):
                    for hp in range(2):
                        pch = hp
                        units = []
                        if qj == 0 and hp == 0:
                            units.append(None)
                            units.append(None)
                            units.append(None)
                            units.append(lambda: emit_qproj(0, 1))
                        elif qj == 0 and hp == 1:
                            units.append(lambda: emit_qproj(1, 0))
                            units.append(None)
                            units.append(None)
                            units.append(lambda: emit_qproj(1, 1))
                        elif hp == 0 and qj <= 2:
                            units.append(lambda q=qj: emit_qproj(q + 1, 0))
                            units.append(
                                lambda q=qj - 1: emit_outproj_pair(q, 0))
                            units.append(lambda q=qj: emit_qproj(q + 1, 1))
                        elif hp == 1 and qj <= 2:
                            units.append(
                                lambda q=qj - 1: emit_outproj_pair(q, 2))
                            units.append(
                                lambda q=qj - 1: emit_outproj_pair(q, 4))
                            units.append(
                                lambda q=qj - 1: emit_outproj_pair(q, 6))
                        elif hp == 0 and qj == 3:
                            units.append(
                                lambda: emit_outproj_pair(2, 0))
                            units.append(
                                lambda: emit_outproj_pair(2, 2))
                        elif hp == 1 and qj == 3:
                            units.append(
                                lambda: emit_outproj_pair(2, 4))
                            units.append(
                                lambda: emit_outproj_pair(2, 6))

                        ctxs = [cxp.tile([65, 512], f32, name=f"ctx{hh}",
                                         bufs=1)
                                for hh in range(2)]
                        pend = None
                        for g in range(8):
                            cur = []
                            for hh in range(2):
                                off = 64 * hh
                                sup = scp.tile([128, 2, 512], f32, name="sup",
                                               bufs=2)
                                for j in range(2):
                                    ki = 2 * g + j
                                    nc.tensor.matmul(
                                        sup[:, j, :],
                                        kT[pch][off:off + 64,
                                                128 * ki:128 * (ki + 1)],
                                        qT[pch][off:off + 64,
                                                512 * qj:512 * (qj + 1)],
                                        start=True, stop=True,
                                        tile_position=(off, 0))
                                ex = expool.tile([128, 2, 512], f16,
                                                 name="ex", bufs=6)
                                nc.scalar.activation(ex[:], sup[:], EXP,
                                                     bias=zbias[:],
                                                     scale=0.125)
                                cur.append(ex)
                            if qj == 0:
                                # two transposes per slot, one slot ahead of
                                # the attn@V that consumes them (pch = hp);
                                # slot g7 makes sc14,15 for the deferred
                                # trailing attn@V of this head pair
                                sc0 = 0 if g == 0 else 2 * g
                                emit_vi(sc0, hp)
                                emit_vi(sc0 + 1, hp)
                            if g == 0 and trailing is not None:
                                trail_p0(trailing)
                            if g == 1 and trailing is not None:
                                trail_p1(trailing)
                                trailing = None
                            if pend is not None:
                                for hh in range(2):
                                    for j in range(2):
                                        pk = 2 * (g - 1) + j
                                        nc.tensor.matmul(
                                            ctxs[hh][:],
                                            vI[pk][:, 2 * pch + hh, :],
                                            pend[hh][:, j, :],
                                            start=(pk == 0), stop=False)
                            pend = cur
                            if g >= 2 and units:
                                u = units.pop(0)
                                if u is not None:
                                    u()
                        trailing = (qj, pch, ctxs, pend, [], [])
                  # drain the last head-pair inside the scores/ctx pools
                  trail_p0(trailing)
                  trail_p1(trailing)
                # final q-slice outproj in a deeper psum pool (scores/ctx
                # banks are free now), so the drain pipeline isn't 2-deep
                with tc.tile_pool(name="drp", bufs=1, space="PSUM") as drp:
                    for m in range(0, 8, 2):
                        emit_outproj_pair(3, m, pool=drp, bufs=4,
                                          use_act=(m >= 4))

    nc.compile()
    return nc


def _make_in_maps(q, k, v, wq, bq, wk, bk, wv, bv, wo):
    f16 = np.float16
    in_maps = []
    for c in range(8):
        b, g = divmod(c, 4)
        fs = slice(FG * g, FG * (g + 1))
        in_maps.append({
            "xq": np.ascontiguousarray(q[b].T.astype(f16)),
            "xk": np.ascontiguousarray(k[b].T.astype(f16)),
            "xv": np.ascontiguousarray(v[b].T.astype(f16)),
            "wq": np.ascontiguousarray(wq[fs, :].T.astype(f16)),
            "wk": np.ascontiguousarray(wk[fs, :].T.astype(f16)),
            "wv": np.ascontiguousarray(wv[fs, :].T.astype(f16)),
            "wo": np.ascontiguousarray(wo[:, fs].T.astype(f16)),
            "bq": np.ascontiguousarray(bq[fs].reshape(FG, 1).astype(np.float32)),
            "bk": np.ascontiguousarray(bk[fs].reshape(FG, 1).astype(np.float32)),
            "bv": np.ascontiguousarray(bv[fs].reshape(FG, 1).astype(np.float32)),
        })
    return in_maps


def kernel(q, k, v, wq, bq, wk, bk, wv, bv, wo, bo):
    from concourse.bass_utils import run_bass_kernel_spmd

    global _compiled
    if _compiled is None:
        _compiled = _build_program()
    nc = _compiled

    args = [np.asarray(a, dtype=np.float32)
            for a in (q, k, v, wq, bq, wk, bk, wv, bv, wo)]
    bo = np.asarray(bo, dtype=np.float32)
    in_maps = _make_in_maps(*args)
    res = run_bass_kernel_spmd(nc, in_maps, core_ids=list(range(8)))
    outs = [np.asarray(res.results[c]["out"], dtype=np.float32)
            for c in range(8)]
    full = []
    for b in range(B):
        acc = outs[4 * b] + outs[4 * b + 1] + outs[4 * b + 2] + outs[4 * b + 3]
        full.append(acc.T + bo[None, :])
    return np.stack(full).astype(np.float32)


# revision 44
# speedup vs baseline: 1.1244x; 1.1244x over previous
"""MultiHeadAttention (B=2, S=2048, D=1024, H=16, depth=64) on 8 trn2 cores.

Sharding: core c -> batch b=c//4, head-group g=c%4 (heads 4g..4g+3).
Per-core device program (SPMD, identical program, different inputs):
  - inputs pre-transposed AND cast to fp16 on host: xq/xk/xv = x_b.T
    [1024, 2048]; weights fp16; biases fp32 column vectors [256, 1].
  - DMA order xv, xk, xq (serial ~360GB/s pipe): V projection first so
    the seq-major V tiles exist before attention starts; K next; Q is
    projected per 512-column q-slice ([128, 8, 512] DMA views), with
    slice 0 / head-pair 0 done up front and the rest emitted INSIDE
    the attention loop (hidden in the ACT exp shadow).
  - V/K projections feature-major, chunk-paced kk-outer over 8 psum
    banks; per-partition bias folded into the DVE psum->SBUF copy.
    DVE copy order: V, K-pch0, Q0-pch0, K-pch1 -- so the first scores
    and the attention psum pools (which reuse K's banks) unblock in
    the order the pipeline needs them.
  - V transposed on-device (PE transpose into the output-projection
    psum slots) and interleaved by DVE into seq-major vI[sc]
    [128, 4, 65] tiles with an all-ones column 64 per head, so attn@V
    also produces the softmax denominator (row 64 of ctx psum).  The
    first 6 seq-blocks are done before the g-loop, the rest ride the
    per-g fill-unit slots of q-slice 0.
  - attention processes head PAIRS with a one-g software-pipeline skew:
    scores(g) for both heads issue before attn@V(g-1); the last attn@V
    + normalization of each head-pair is DEFERRED past the next pair's
    first scores so the ACT exp stream never gaps at boundaries.  exp
    on ACT (scale 1/8, no max subtraction: scores ~ N(0,1)); ACT does
    nothing except exp (table preloaded at t=0).
  - normalization: reciprocal of ctx row 64 (DVE, f32r) -> rank-1 PE
    matmul broadcast -> multiply (DVE) into feature-major fp16 ctxN.
  - output projection of q-slice j spread over fill-unit slots inside
    q-slice j+1's attention; psum -> fp16 SBUF (DVE) in m-block pairs,
    one DMA per pair.  Host sums the 4 head-group partials per batch
    in f32, transposes back, adds bo.
"""

import numpy as np

B, S, D = 2, 2048, 1024
FG = 256  # features per core (4 heads x 64)

_compiled = None


def _build_program(repeat=1):
    import concourse.bass as bass  # noqa: F401
    import concourse.tile as tile
    from concourse import bacc, mybir, masks

    f32 = mybir.dt.float32
    f32r = mybir.dt.float32r
    f16 = mybir.dt.float16
    EXP = mybir.ActivationFunctionType.Exp
    MULT = mybir.AluOpType.mult

    nc = bacc.Bacc("TRN2", target_bir_lowering=False, debug=False)

    xq_d = nc.dram_tensor("xq", [D, S], f16, kind="ExternalInput")
    xk_d = nc.dram_tensor("xk", [D, S], f16, kind="ExternalInput")
    xv_d = nc.dram_tensor("xv", [D, S], f16, kind="ExternalInput")
    wq_d = nc.dram_tensor("wq", [D, FG], f16, kind="ExternalInput")
    wk_d = nc.dram_tensor("wk", [D, FG], f16, kind="ExternalInput")
    wv_d = nc.dram_tensor("wv", [D, FG], f16, kind="ExternalInput")
    wo_d = nc.dram_tensor("wo", [FG, D], f16, kind="ExternalInput")
    bq_d = nc.dram_tensor("bq", [FG, 1], f32, kind="ExternalInput")
    bk_d = nc.dram_tensor("bk", [FG, 1], f32, kind="ExternalInput")
    bv_d = nc.dram_tensor("bv", [FG, 1], f32, kind="ExternalInput")
    out_d = nc.dram_tensor("out", [D, S], f16, kind="ExternalOutput")

    with tile.TileContext(nc) as tc:
      for _rep in range(repeat):
        with tc.tile_pool(name="const", bufs=1) as cpool:
            onesf = cpool.tile([1, 64], f32, tag="onesf", name="onesf")
            nc.gpsimd.memset(onesf[:], 1.0)
            ones_r = cpool.tile([1, 64], f32r, tag="ones_r", name="ones_r")
            nc.vector.tensor_copy(ones_r[:], onesf[:])
            o41f = cpool.tile([128, 4, 1], f32, tag="o41f", name="o41f")
            nc.gpsimd.memset(o41f[:], 1.0)
            ones41 = cpool.tile([128, 4, 1], f16, tag="ones41", name="ones41")
            nc.gpsimd.tensor_copy(ones41[:], o41f[:])
            zbias = cpool.tile([128, 1], f32, tag="zbias", name="zbias")
            nc.gpsimd.memset(zbias[:], 0.0)
            # preload the ACT exp table while projections run
            actwarm = cpool.tile([128, 1], f16, tag="actwarm", name="actwarm")
            nc.scalar.activation(actwarm[:], zbias[:], EXP, bias=zbias[:],
                                 scale=1.0)
            ident = cpool.tile([128, 128], f16, tag="ident", name="ident")
            masks.make_identity(nc, ident[:])
            dumf = cpool.tile([1, 512], f32, tag="dumf", name="dumf")
            nc.gpsimd.memset(dumf[:], 1.0)
            dum_r = cpool.tile([1, 512], f32r, tag="dum_r", name="dum_r")
            nc.vector.tensor_copy(dum_r[:], dumf[:])

            w_sb = {}
            b_sb = {}

            def load_w(nm, d):
                t = cpool.tile([128, 8, FG], f16, tag=nm, name=nm)
                nc.sync.dma_start(t[:], d.ap().rearrange("(k p) f -> p k f", p=128))
                w_sb[nm] = t

            def load_b(nm, d):
                t = cpool.tile([128, 2, 1], f32, tag=nm, name=nm)
                nc.sync.dma_start(t[:], d.ap().rearrange("(k p) o -> p k o", p=128))
                b_sb[nm] = t

            qT = [cpool.tile([128, S], f16, tag=f"qT{p}", name=f"qT{p}")
                  for p in range(2)]
            kT = [cpool.tile([128, S], f16, tag=f"kT{p}", name=f"kT{p}")
                  for p in range(2)]
            vT = [cpool.tile([128, S], f16, tag=f"vT{p}", name=f"vT{p}")
                  for p in range(2)]
            vI = [cpool.tile([128, 4, 65], f16, tag=f"vI{sc}", name=f"vI{sc}")
                  for sc in range(16)]
            for sc in range(16):
                nc.gpsimd.tensor_copy(vI[sc][:, :, 64:65], ones41[:])
            ctxN = [cpool.tile([128, S], f16, tag=f"ctxN{p}", name=f"ctxN{p}")
                    for p in range(2)]

            # ------------- V and K projections (8 psum banks, kk-outer) ------
            with tc.tile_pool(name="xp", bufs=1) as xpool, \
                 tc.tile_pool(name="pp", bufs=1, space="PSUM") as ppool:

                def proj_matmuls(x_d, wname):
                    ps = [ppool.tile([128, 512], f32, name=f"pp{i}", bufs=1)
                          for i in range(8)]
                    xss = []
                    for c in range(4):
                        xs = xpool.tile([128, 2, S], f16, name="xs", bufs=8)
                        nc.sync.dma_start(
                            xs[:],
                            x_d.ap()[256 * c:256 * (c + 1), :]
                                .rearrange("(j p) s -> p j s", p=128))
                        xss.append(xs)
                    for kk in range(8):
                        for pch in range(2):
                            for qc in range(4):
                                i = pch * 4 + qc
                                nc.tensor.matmul(
                                    ps[i][:],
                                    w_sb[wname][:, kk, 128 * pch:128 * (pch + 1)],
                                    xss[kk // 2][:, kk % 2,
                                                 512 * qc:512 * (qc + 1)],
                                    start=(kk == 0), stop=(kk == 7))
                    return ps

                def proj_copies(ps, bname, outT, pch):
                    for qc in range(4):
                        nc.vector.tensor_scalar_add(
                            outT[pch][:, 512 * qc:512 * (qc + 1)],
                            ps[pch * 4 + qc][:], b_sb[bname][:, pch, :])

                def xq_dma(qj):
                    xs = xpool.tile([128, 8, 512], f16, name=f"xq{qj}", bufs=1)
                    nc.sync.dma_start(
                        xs[:],
                        xq_d.ap()[:, 512 * qj:512 * (qj + 1)]
                            .rearrange("(k p) s -> p k s", p=128))
                    return xs

                xqs = []

                def project_q_pch(qj, pch, pool_tile):
                    for kk in range(8):
                        nc.tensor.matmul(
                            pool_tile[:],
                            w_sb["wq"][:, kk, 128 * pch:128 * (pch + 1)],
                            xqs[qj][:, kk, :],
                            start=(kk == 0), stop=(kk == 7))
                    nc.vector.tensor_scalar_add(
                        qT[pch][:, 512 * qj:512 * (qj + 1)],
                        pool_tile[:], b_sb["bq"][:, pch, :])

                # PE p-state warmup: dummy rank-1 matmuls keep the PE
                # ramping until the first xk chunk lands
                warm = ppool.tile([64, 512], f32, name="pp7", bufs=1)
                for _w in range(12):
                    nc.tensor.matmul(warm[:], ones_r[:, 0:64], dum_r[:],
                                     start=True, stop=True)
                # wire: wk,bk, xk, wq,bq, xq0, wv,bv, xv, wo, xq1-3
                load_w("wk", wk_d)
                load_b("bk", bk_d)
                ps = proj_matmuls(xk_d, "wk")
                load_w("wq", wq_d)
                load_b("bq", bq_d)
                xqs.append(xq_dma(0))
                proj_copies(ps, "bk", kT, 0)
                # q-slice 0 / head-pair 0 right after K
                psq = ppool.tile([128, 512], f32, name="pp0", bufs=1)
                project_q_pch(0, 0, psq)
                load_w("wv", wv_d)
                load_b("bv", bv_d)
                proj_copies(ps, "bk", kT, 1)
                vps = proj_matmuls(xv_d, "wv")
                wo_sb = cpool.tile([128, 2, D], f16, tag="wo", name="wo")
                nc.sync.dma_start(wo_sb[:],
                                  wo_d.ap().rearrange("(k p) f -> p k f", p=128))
                for qj in range(1, 4):
                    xqs.append(xq_dma(qj))

                # drain V psum on DVE and ACT in parallel: the attention
                # psum pools wait for the whole 8-bank region
                IDENT_F = mybir.ActivationFunctionType.Identity
                for i, (pch, qc) in enumerate(
                        ((0, 0), (0, 1), (1, 0), (1, 1),
                         (0, 2), (0, 3), (1, 2), (1, 3))):
                    if i % 2 == 0:
                        nc.vector.tensor_scalar_add(
                            vT[pch][:, 512 * qc:512 * (qc + 1)],
                            vps[pch * 4 + qc][:], b_sb["bv"][:, pch, :])
                    else:
                        nc.scalar.activation(
                            vT[pch][:, 512 * qc:512 * (qc + 1)],
                            vps[pch * 4 + qc][:], IDENT_F,
                            bias=b_sb["bv"][:, pch, :], scale=1.0)

            # ---------------- attention + output projection ------------------
            with tc.tile_pool(name="opp", bufs=2, space="PSUM") as opp, \
                 tc.tile_pool(name="exp", bufs=8) as expool, \
                 tc.tile_pool(name="rcp", bufs=2) as rcpool, \
                 tc.tile_pool(name="csp", bufs=2) as cspool, \
                 tc.tile_pool(name="obp", bufs=4) as obpool:

                def emit_outproj_pair(qj, m, pool=None, bufs=2,
                                      use_act=False):
                    pool = pool or opp
                    ops = []
                    for mm in (m, m + 1):
                        op = pool.tile([128, 512], f32, name="op", bufs=bufs)
                        for kk2 in range(2):
                            nc.tensor.matmul(
                                op[:],
                                wo_sb[:, kk2, 128 * mm:128 * (mm + 1)],
                                ctxN[kk2][:, 512 * qj:512 * (qj + 1)],
                                start=(kk2 == 0), stop=(kk2 == 1))
                        ops.append(op)
                    ob = obpool.tile([128, 2, 512], f16, name="ob", bufs=4)
                    for t in range(2):
                        if use_act:
                            nc.scalar.activation(ob[:, t, :], ops[t][:],
                                                 mybir.ActivationFunctionType.Copy,
                                                 bias=0.0, scale=1.0)
                        else:
                            nc.vector.tensor_copy(ob[:, t, :], ops[t][:])
                    nc.sync.dma_start(
                        out_d.ap()[128 * m:128 * (m + 2),
                                   512 * qj:512 * (qj + 1)]
                            .rearrange("(t p) s -> p t s", p=128),
                        ob[:])

                def emit_qproj(qj, pch):
                    pt = opp.tile([128, 512], f32, name="op", bufs=2)
                    project_q_pch(qj, pch, pt)

                def emit_vi(sc, pch):
                    tp = opp.tile([128, 128], f16, name="op", bufs=2)
                    nc.tensor.transpose(
                        tp[:], vT[pch][:, 128 * sc:128 * (sc + 1)],
                        ident[:])
                    nc.vector.tensor_copy(
                        vI[sc][:, 2 * pch:2 * pch + 2, 0:64],
                        tp[:].rearrange("p (h e) -> p h e", h=2))

                # trailing work deferred across head-pair boundaries:
                # phase 0 = last attn@V + ctx staging; phase 1 = normalize
                def trail_p0(st):
                    qj, pch, ctxs, pend, css, rcs = st
                    for hh in range(2):
                        for j in range(2):
                            pk = 14 + j
                            nc.tensor.matmul(
                                ctxs[hh][:], vI[pk][:, 2 * pch + hh, :],
                                pend[hh][:, j, :],
                                start=False, stop=(pk == 15))
                    for hh in range(2):
                        cs = cspool.tile([64, 512], f32, name="cs", bufs=4)
                        nc.vector.tensor_copy(cs[:], ctxs[hh][0:64, :])
                        css.append(cs)
                        rc = rcpool.tile([1, 512], f32r, name="rc", bufs=4)
                        with nc.allow_low_precision(
                                reason="f32r for PE broadcast"):
                            nc.vector.reciprocal(rc[:], ctxs[hh][64:65, :])
                        rcs.append(rc)

                def trail_p1(st):
                    qj, pch, ctxs, pend, css, rcs = st
                    for hh in range(2):
                        off = 64 * hh
                        bc = opp.tile([64, 512], f32, name="op", bufs=2)
                        nc.tensor.matmul(bc[:], ones_r[:, :], rcs[hh][:],
                                         start=True, stop=True)
                        nc.vector.tensor_tensor(
                            ctxN[pch][off:off + 64, 512 * qj:512 * (qj + 1)],
                            css[hh][:], bc[:], MULT)

                trailing = None
                with tc.tile_pool(name="scp", bufs=1, space="PSUM") as scp, \
                     tc.tile_pool(name="cxp", bufs=1, space="PSUM") as cxp:
                  for qj in range(4):
                    for hp in range(2):
                        pch = hp
                        units = []
                        if qj == 0 and hp == 0:
                            units.append(None)
                            units.append(None)
                            units.append(None)
                            units.append(lambda: emit_qproj(0, 1))
                        elif qj == 0 and hp == 1:
                            units.append(lambda: emit_qproj(1, 0))
                            units.append(None)
                            units.append(None)
                            units.append(lambda: emit_qproj(1, 1))
                        elif hp == 0 and qj <= 2:
                            units.append(lambda q=qj: emit_qproj(q + 1, 0))
                            units.append(
                                lambda q=qj - 1: emit_outproj_pair(q, 0))
                            units.append(lambda q=qj: emit_qproj(q + 1, 1))
                        elif hp == 1 and qj <= 2:
                            units.append(
                                lambda q=qj - 1: emit_outproj_pair(q, 2))
                            units.append(
                                lambda q=qj - 1: emit_outproj_pair(q, 4))
                            units.append(
                                lambda q=qj - 1: emit_outproj_pair(q, 6))
                        elif hp == 0 and qj == 3:
                            units.append(
                                lambda: emit_outproj_pair(2, 0))
                            units.append(
                                lambda: emit_outproj_pair(2, 2))
                        elif hp == 1 and qj == 3:
                            units.append(
                                lambda: emit_outproj_pair(2, 4))
                            units.append(
                                lambda: emit_outproj_pair(2, 6))

                        ctxs = [cxp.tile([65, 512], f32, name=f"ctx{hh}",
                                         bufs=1)
                                for hh in range(2)]
                        pend = None
                        for g in range(8):
                            cur = []
                            for hh in range(2):
                                off = 64 * hh
                                sup = scp.tile([128, 2, 512], f32, name="sup",
                                               bufs=2)
                                for j in range(2):
                                    ki = 2 * g + j
                                    nc.tensor.matmul(
                                        sup[:, j, :],
                                        kT[pch][off:off + 64,
                                                128 * ki:128 * (ki + 1)],
                                        qT[pch][off:off + 64,
                                                512 * qj:512 * (qj + 1)],
                                        start=True, stop=True,
                                        tile_position=(off, 0))
                                ex = expool.tile([128, 2, 512], f16,
                                                 name="ex", bufs=6)
                                nc.scalar.activation(ex[:], sup[:], EXP,
                                                     bias=zbias[:],
                                                     scale=0.125)
                                cur.append(ex)
                            if qj == 0:
                                # two transposes per slot, one slot ahead of
                                # the attn@V that consumes them (pch = hp);
                                # slot g7 makes sc14,15 for the deferred
                                # trailing attn@V of this head pair
                                sc0 = 0 if g == 0 else 2 * g
                                emit_vi(sc0, hp)
                                emit_vi(sc0 + 1, hp)
                            if g == 0 and trailing is not None:
                                trail_p0(trailing)
                            if g == 1 and trailing is not None:
                                trail_p1(trailing)
                                trailing = None
                            if pend is not None:
                                for hh in range(2):
                                    for j in range(2):
                                        pk = 2 * (g - 1) + j
                                        nc.tensor.matmul(
                                            ctxs[hh][:],
                                            vI[pk][:, 2 * pch + hh, :],
                                            pend[hh][:, j, :],
                                            start=(pk == 0), stop=False)
                            pend = cur
                            if g >= 2 and units:
                                u = units.pop(0)
                                if u is not None:
                                    u()
                        trailing = (qj, pch, ctxs, pend, [], [])
                  # drain the last head-pair inside the scores/ctx pools
                  trail_p0(trailing)
                  trail_p1(trailing)
                # final q-slice outproj in a deeper psum pool (scores/ctx
                # banks are free now), so the drain pipeline isn't 2-deep
                with tc.tile_pool(name="drp", bufs=1, space="PSUM") as drp:
                    for m in range(0, 8, 2):
                        emit_outproj_pair(3, m, pool=drp, bufs=4,
                                          use_act=(m >= 4))

    nc.compile()
    return nc


def _make_in_maps(q, k, v, wq, bq, wk, bk, wv, bv, wo):
    f16 = np.float16
    in_maps = []
    for c in range(8):
        b, g = divmod(c, 4)
        fs = slice(FG * g, FG * (g + 1))
        in_maps.append({
            "xq": np.ascontiguousarray(q[b].T.astype(f16)),
            "xk": np.ascontiguousarray(k[b].T.astype(f16)),
            "xv": np.ascontiguousarray(v[b].T.astype(f16)),
            "wq": np.ascontiguousarray(wq[fs, :].T.astype(f16)),
            "wk": np.ascontiguousarray(wk[fs, :].T.astype(f16)),
            "wv": np.ascontiguousarray(wv[fs, :].T.astype(f16)),
            "wo": np.ascontiguousarray(wo[:, fs].T.astype(f16)),
            "bq": np.ascontiguousarray(bq[fs].reshape(FG, 1).astype(np.float32)),
            "bk": np.ascontiguousarray(bk[fs].reshape(FG, 1).astype(np.float32)),
            "bv": np.ascontiguousarray(bv[fs].reshape(FG, 1).astype(np.float32)),
        })
    return in_maps


def kernel(q, k, v, wq, bq, wk, bk, wv, bv, wo, bo):
    from concourse.bass_utils import run_bass_kernel_spmd

    global _compiled
    if _compiled is None:
        _compiled = _build_program()
    nc = _compiled

    args = [np.asarray(a, dtype=np.float32)
            for a in (q, k, v, wq, bq, wk, bk, wv, bv, wo)]
    bo = np.asarray(bo, dtype=np.float32)
    in_maps = _make_in_maps(*args)
    res = run_bass_kernel_spmd(nc, in_maps, core_ids=list(range(8)))
    outs = [np.asarray(res.results[c]["out"], dtype=np.float32)
            for c in range(8)]
    full = []
    for b in range(B):
        acc = outs[4 * b] + outs[4 * b + 1] + outs[4 * b + 2] + outs[4 * b + 3]
        full.append(acc.T + bo[None, :])
    return np.stack(full).astype(np.float32)
